# revision 1
# baseline (speedup 1.0000x reference)
"""BDH dense-transformer Trainium2 kernel (8 NeuronCores, SPMD).

Model (weight-tied, 4 layers): T=1024, D=256, NH=4, N=8192/head, VOCAB=256.

Sharding: core c -> head h=c//2, latent half j=c%2 (4096 latent dims/core).
  - encoder/encoder_v column-sharded, decoder row-sharded (host-permuted so
    rope pairs are de-interleaved: local m in [0,2048) = even pair elements,
    [2048,4096) = odd elements; permutation applied consistently to all three
    weight shards so scores/gate/decoder are unaffected).
  - scores trick: yKV = mask(qr qr^T) @ x distributes over latent shards:
    each core computes mask(qr_loc qr_loc^T) @ x, pairwise AllReduce on yKV.
  - decoder partial sums: 8-way AllReduce on y.

On-device layouts: big tensors feature-major [latent, T] (contraction dims on
partitions); residual stream x token-major [T, D] fp32; matmuls bf16 with
fp32 PSUM accumulation.
"""

import os
import numpy as np
import ml_dtypes

BF16NP = ml_dtypes.bfloat16

# full-size config
T = 1024
D = 256
NH = 4
N = 8192
V = 256
L = 4
NCORES = 8
P = 128
EPS = 1e-5
THETA = 2.0 ** 16

_CACHE = {}


def _cfg(n_cores=NCORES, half=N // 2, t=T, layers=L, no_cc=False):
    # half: latent width per core (= N*NH/n_cores)
    assert t % 512 == 0 and half % 256 == 0
    return dict(
        n_cores=n_cores, half=half, t=t, layers=layers, no_cc=no_cc,
        tch=t // P,            # token chunks
        kch=half // P,         # latent chunks
        pblk=half // 2 // P,   # pair blocks (even/odd chunk pairs)
        tb_n=t // 512,         # 512-wide t blocks
        dch=D // P,            # 2
    )


# ---------------------------------------------------------------- device code

def emit_model(tc, in_aps, out_ap, cfg):
    from contextlib import ExitStack
    import concourse.mybir as mybir
    from concourse.masks import make_identity

    from concourse.bass import ds

    nc = tc.nc
    f32 = mybir.dt.float32
    bf = mybir.dt.bfloat16
    ADD = mybir.AluOpType.add
    SUB = mybir.AluOpType.subtract
    MULT = mybir.AluOpType.mult
    Relu = mybir.ActivationFunctionType.Relu
    Sqrt = mybir.ActivationFunctionType.Sqrt

    n_cores = cfg["n_cores"]
    TCH, KCH, PBLK, TB, DCH = (cfg[k] for k in ("tch", "kch", "pblk", "tb_n", "dch"))
    TT = cfg["t"]
    HALF = cfg["half"]
    layers = cfg["layers"]
    pair_groups = [[2 * i, 2 * i + 1] for i in range(n_cores // 2)]
    all_group = [list(range(n_cores))]

    with ExitStack() as ctx:
        singles = ctx.enter_context(tc.tile_pool(name="singles", bufs=1))
        state = ctx.enter_context(tc.tile_pool(name="state", bufs=1))
        dram = ctx.enter_context(tc.tile_pool(name="dramp", bufs=1, space="DRAM"))
        lnp = ctx.enter_context(tc.tile_pool(name="lnp", bufs=4))

        # ---- resident constants (x0 is DMA'd first, below; w_e next: phase B
        # depends only on those, and the x0 LN chain overlaps the w_e DMA)
        we_sb = singles.tile([P, DCH, HALF], bf, name="we_sb")
        nc.sync.dma_start(out=we_sb[:], in_=in_aps["w_e"].rearrange("(ko p) m -> p ko m", p=P))
        mask_sb = singles.tile([P, P], f32, name="mask_sb")
        nc.gpsimd.dma_start(out=mask_sb[:], in_=in_aps["mask"][:])
        wv_sb = singles.tile([P, DCH, HALF], bf, name="wv_sb")
        nc.gpsimd.dma_start(out=wv_sb[:], in_=in_aps["w_v"].rearrange("(ko p) m -> p ko m", p=P))
        dec_sb = singles.tile([P, KCH, D], bf, name="dec_sb")
        nc.gpsimd.dma_start(out=dec_sb[:], in_=in_aps["dec"].rearrange("(kc p) d -> p kc d", p=P))
        lm_sb = singles.tile([P, DCH, V], bf, name="lm_sb")
        nc.gpsimd.dma_start(out=lm_sb[:], in_=in_aps["lm"].rearrange("(ko p) v -> p ko v", p=P))
        eps_sb = singles.tile([P, 1], f32, name="eps_sb")
        nc.vector.memset(eps_sb[:], EPS)
        id_bf = singles.tile([P, P], bf, name="id_bf")
        make_identity(nc, id_bf[:])
        id_f32 = singles.tile([P, P], f32, name="id_f32")
        make_identity(nc, id_f32[:])

        # ---- resident state
        x_sb = state.tile([P, TCH, D], f32, name="x_sb")        # residual, token-major
        tmaj_bf = state.tile([P, TCH, D], bf, name="tmaj_bf")   # xb
        ykvln_bf = state.tile([P, TCH, D], bf, name="ykvln_bf")  # LN(yKV) bf16
        dmaj_bf = state.tile([P, DCH, TT], bf, name="dmaj_bf")  # xT / yKV_lnT
        qr_sb = state.tile([P, KCH, TT], bf, name="qr_sb")
        td_f32 = state.tile([P, TCH, D], f32, name="td_f32")    # x0 / yKV / y (token-major)
        yT_sb = state.tile([P, DCH, TT], f32, name="yT_sb")     # y feature-major

        # ---- dram scratch
        xsp_dram = dram.tile([P, KCH, TT], bf, name="xsp_dram")
        ar1_ins = [dram.tile([P, 4, D], f32, name=f"ar1_in{i}") for i in range(TB)]
        ar1_outs = [dram.tile([P, 4, D], f32, name=f"ar1_out{i}") for i in range(TB)]
        CHT = TCH // max(n_cores, 1)  # token chunks per rank after ReduceScatter
        ar2_in = dram.tile([TCH, P, D], f32, name="ar2_in")
        ar2rs_out = dram.tile([max(CHT, 1), P, D], f32, name="ar2rs_out")
        ar3_in = dram.tile([max(CHT, 1), P, D], f32, name="ar3_in")
        ar3_out = dram.tile([TCH, P, D], f32, name="ar3_out")

        def emit_ln_2d(src2, dst2):
            # LayerNorm over D of one [P, D] tile
            stats = lnp.tile([P, 6], f32, name="ln_stats", tag="ln_stats")
            nc.vector.bn_stats(out=stats[:], in_=src2)
            mv = lnp.tile([P, 2], f32, name="ln_mv", tag="ln_mv")
            nc.vector.bn_aggr(out=mv[:], in_=stats[:])
            std = lnp.tile([P, 1], f32, name="ln_std", tag="ln_std")
            nc.scalar.activation(out=std[:], in_=mv[:, 1:2], func=Sqrt,
                                 bias=eps_sb[:, 0:1])
            rstd = lnp.tile([P, 1], f32, name="ln_rstd", tag="ln_rstd")
            nc.vector.reciprocal(out=rstd[:], in_=std[:])
            nc.vector.tensor_scalar(out=dst2, in0=src2,
                                    scalar1=mv[:, 0:1], scalar2=rstd[:],
                                    op0=SUB, op1=MULT)

        def emit_ln(src3, dst3, taus=None):
            # LayerNorm over D for each token chunk; src/dst [P, TCH, D]
            for tau in (range(TCH) if taus is None else taus):
                emit_ln_2d(src3[:, tau, :], dst3[:, tau, :])

        def emit_transpose(ps_pool, src2, dst2, ident, dtype, tag):
            pt = ps_pool.tile([P, P], dtype, name=f"tp_{tag}", tag=f"tp_{tag}")
            nc.tensor.transpose(pt[:], src2, ident[:])
            nc.vector.tensor_copy(out=dst2, in_=pt[:])

        def phase_A(tpps):
            # xb = bf16(x); xT = transpose(xb)
            nc.vector.tensor_copy(out=tmaj_bf[:], in_=x_sb[:])
            for tau in range(TCH):
                for ko in range(DCH):
                    emit_transpose(tpps, tmaj_bf[:, tau, ko * P:(ko + 1) * P],
                                   dmaj_bf[:, ko, tau * P:(tau + 1) * P], id_bf, bf, "a")

        # ---- initial: x = LN(x0)  (x0 DMA races ahead of the weight loads)
        pid = nc.vector.partition_id() if n_cores > 1 else None
        nc.scalar.dma_start(out=td_f32[:], in_=in_aps["x0"].rearrange("(tau p) d -> p tau d", p=P))
        emit_ln(td_f32, x_sb)

        for layer in range(layers):
            # ---------------- phase A: xb, xT
            with tc.tile_pool(name=f"tpA{layer}", bufs=2, space="PSUM") as tpps:
                phase_A(tpps)

            # ---------------- phase B: x_sp = relu(W_e^T xT); rope -> qr; spill x_sp.
            # The tb=0 score tiles accumulate IN phase B with a 2-block lag:
            # scores(si,tb=0,kc) only needs qr chunk kc, so PE fills its
            # rope-wait bubbles with early score accumulation.
            scB_cm = tc.tile_pool(name=f"scB{layer}", bufs=1, space="PSUM")
            scB = scB_cm.__enter__()
            sc0_ps = [scB.tile([P, 512], f32, name=f"sc0_{si}", tag=f"sc0_{si}",
                               bufs=1) for si in range(4)]

            def emit_sc0(blk):
                # accumulate chunk pair (blk, blk+PBLK) into tb=0 tiles
                for si in range(4):
                    for kc in (blk, blk + PBLK):
                        nc.tensor.matmul(
                            sc0_ps[si][:], lhsT=qr_sb[:, kc, si * P:(si + 1) * P],
                            rhs=qr_sb[:, kc, 0:512],
                            start=(blk == 0 and kc == blk),
                            stop=(blk == PBLK - 1 and kc == blk + PBLK),
                            skip_group_check=True)

            LAG = 2
            with tc.tile_pool(name=f"pb{layer}", bufs=3) as work, \
                 tc.tile_pool(name=f"pbps{layer}", bufs=4, space="PSUM") as ps:
                for blk in range(PBLK + LAG):
                  if blk < PBLK:
                    cos_sb = work.tile([P, TT], bf, name="cos_sb", tag="cos")
                    nc.sync.dma_start(out=cos_sb[:], in_=in_aps["cos_t"][blk * P:(blk + 1) * P, :])
                    sin_sb = work.tile([P, TT], bf, name="sin_sb", tag="sin")
                    nc.sync.dma_start(out=sin_sb[:], in_=in_aps["sin_t"][blk * P:(blk + 1) * P, :])
                    ve = work.tile([P, TT], bf, name="ve", tag="ve")
                    vo = work.tile([P, TT], bf, name="vo", tag="vo")
                    for parity, vt in ((0, ve), (1, vo)):
                        kc = blk + PBLK * parity
                        for tb in range(TB):
                            pt = ps.tile([P, 512], f32, name="xsp_ps", tag="xsp_ps")
                            for ko in range(DCH):
                                nc.tensor.matmul(
                                    pt[:], lhsT=we_sb[:, ko, kc * P:(kc + 1) * P],
                                    rhs=dmaj_bf[:, ko, tb * 512:(tb + 1) * 512],
                                    start=(ko == 0), stop=(ko == DCH - 1))
                            nc.scalar.activation(out=vt[:, tb * 512:(tb + 1) * 512],
                                                 in_=pt[:], func=Relu)
                    nc.sync.dma_start(out=xsp_dram[:, blk, :], in_=ve[:])
                    nc.sync.dma_start(out=xsp_dram[:, blk + PBLK, :], in_=vo[:])
                    # rope: 5 ops on DVE, 1 combine on GpSimd (load-balanced)
                    t1 = work.tile([P, TT], bf, name="t1", tag="t1")
                    t2 = work.tile([P, TT], bf, name="t2", tag="t2")
                    nc.vector.tensor_mul(out=t1[:], in0=ve[:], in1=cos_sb[:])
                    nc.vector.tensor_mul(out=t2[:], in0=vo[:], in1=sin_sb[:])
                    nc.vector.tensor_tensor(qr_sb[:, blk, :], t1[:], t2[:], SUB)
                    t3 = work.tile([P, TT], bf, name="t3", tag="t1")
                    t4 = work.tile([P, TT], bf, name="t4", tag="t2")
                    nc.vector.tensor_mul(out=t3[:], in0=vo[:], in1=cos_sb[:])
                    nc.vector.tensor_mul(out=t4[:], in0=ve[:], in1=sin_sb[:])
                    nc.gpsimd.tensor_tensor(qr_sb[:, blk + PBLK, :], t3[:], t4[:], ADD)
                  if blk >= LAG:
                    emit_sc0(blk - LAG)

            # ---------------- phases C/D/E pipelined per t-block:
            # scores+yKV(tb) -> AR1(tb) -> LN+transpose(tb); the tb=0 AR/LN
            # overlap the tb=1 scores matmuls.
            with tc.tile_pool(name=f"pc{layer}", bufs=2) as work, \
                 tc.tile_pool(name=f"pcs{layer}", bufs=2, space="PSUM") as scps, \
                 tc.tile_pool(name=f"pcy{layer}", bufs=1, space="PSUM") as ykps, \
                 tc.tile_pool(name=f"tpE{layer}", bufs=1, space="PSUM") as tpps:
                for tb in range(TB):
                    sc_sb = work.tile([P, 4 * (tb + 1), 512], bf, name="sc_sb",
                                      tag=f"sc{tb}", bufs=1)
                    n_s = 4 * (tb + 1)
                    for si in range(n_s):
                        if tb == 0:
                            pt = sc0_ps[si]  # accumulated during phase B
                        else:
                            pt = scps.tile([P, 512], f32, name="sc_ps", tag="sc_ps")
                            for kc in range(KCH):
                                nc.tensor.matmul(
                                    pt[:], lhsT=qr_sb[:, kc, si * P:(si + 1) * P],
                                    rhs=qr_sb[:, kc, tb * 512:(tb + 1) * 512],
                                    start=(kc == 0), stop=(kc == KCH - 1))
                        sub_d = si - 4 * tb
                        if 0 <= sub_d <= 3:
                            nc.vector.tensor_mul(
                                out=sc_sb[:, si, sub_d * P:(sub_d + 1) * P],
                                in0=pt[:, sub_d * P:(sub_d + 1) * P], in1=mask_sb[:])
                            if sub_d < 3:
                                nc.vector.tensor_copy(
                                    out=sc_sb[:, si, (sub_d + 1) * P:],
                                    in_=pt[:, (sub_d + 1) * P:])
                        else:
                            nc.vector.tensor_copy(out=sc_sb[:, si, :], in_=pt[:])
                    for sub in range(4):
                        tau = tb * 4 + sub
                        yk = ykps.tile([P, D], f32, name="yk_ps", tag="yk_ps")
                        for si in range(tau + 1):
                            nc.tensor.matmul(
                                yk[:], lhsT=sc_sb[:, si, sub * P:(sub + 1) * P],
                                rhs=tmaj_bf[:, si, :],
                                start=(si == 0), stop=(si == tau))
                        nc.vector.tensor_copy(out=td_f32[:, tau, :], in_=yk[:])
                    # AR1 for this t-block's yKV partial (overlaps next tb's PE work)
                    taus = range(tb * 4, tb * 4 + 4)
                    if n_cores > 1:
                        nc.sync.dma_start(out=ar1_ins[tb][:],
                                          in_=td_f32[:, tb * 4:(tb + 1) * 4, :])
                        if cfg.get("no_cc"):
                            nc.sync.dma_start(out=ar1_outs[tb][:], in_=ar1_ins[tb][:])
                        else:
                            nc.gpsimd.collective_compute(
                                "AllReduce", ADD, replica_groups=pair_groups,
                                ins=[ar1_ins[tb].opt()], outs=[ar1_outs[tb].opt()])
                        nc.sync.dma_start(out=td_f32[:, tb * 4:(tb + 1) * 4, :],
                                          in_=ar1_outs[tb][:])
                    # LN(yKV) -> bf16 + transpose for this t-block
                    emit_ln(td_f32, ykvln_bf, taus=taus)
                    if tb < TB - 1:
                        # last t-block's transposes are deferred until after
                        # F(0) is emitted, so F(0)'s matmuls fill the PE queue
                        # while AR1b + LN(tb=1) run (avoids head-of-line stall)
                        for tau in taus:
                            for ko in range(DCH):
                                emit_transpose(tpps, ykvln_bf[:, tau, ko * P:(ko + 1) * P],
                                               dmaj_bf[:, ko, tau * P:(tau + 1) * P],
                                               id_bf, bf, "e")
            scB_cm.__exit__(None, None, None)

            # ---------------- phase F: y_sp, gate, decoder partials (per t-block;
            # F(tb) only needs dmaj columns of tb, so F(0) fills the AR1b/LN gap)
            with tc.tile_pool(name=f"pf{layer}", bufs=4) as work, \
                 tc.tile_pool(name=f"pfy{layer}", bufs=1, space="PSUM") as psY, \
                 tc.tile_pool(name=f"pfs{layer}", bufs=2, space="PSUM") as psF, \
                 tc.tile_pool(name=f"tpF{layer}", bufs=1, space="PSUM") as tpf:
                for tb in range(TB):
                    if tb == TB - 1:
                        # deferred transposes of the last t-block's yKV_ln
                        for tau in range((TB - 1) * 4, TB * 4):
                            for ko in range(DCH):
                                emit_transpose(tpf, ykvln_bf[:, tau, ko * P:(ko + 1) * P],
                                               dmaj_bf[:, ko, tau * P:(tau + 1) * P],
                                               id_bf, bf, "e2")
                    yT_ps = [psY.tile([P, 512], f32, name=f"yt_ps{dh}", tag=f"yt_ps{dh}")
                             for dh in range(DCH)]
                    for blk in range(PBLK):
                        for parity in (0, 1):
                            kc = blk + PBLK * parity
                            ysp = work.tile([P, 512], bf, name="ysp", tag="ysp")
                            pt = psF.tile([P, 512], f32, name="ysp_ps", tag="ysp_ps")
                            for ko in range(DCH):
                                nc.tensor.matmul(
                                    pt[:], lhsT=wv_sb[:, ko, kc * P:(kc + 1) * P],
                                    rhs=dmaj_bf[:, ko, tb * 512:(tb + 1) * 512],
                                    start=(ko == 0), stop=(ko == DCH - 1))
                            nc.scalar.activation(out=ysp[:], in_=pt[:], func=Relu)
                            xsp = work.tile([P, 512], bf, name="xsp_r", tag="xsp_r")
                            nc.sync.dma_start(
                                out=xsp[:],
                                in_=xsp_dram[:, kc, tb * 512:(tb + 1) * 512])
                            xy = work.tile([P, 512], bf, name="xy", tag="xy")
                            nc.vector.tensor_mul(out=xy[:], in0=xsp[:], in1=ysp[:])
                            first = (blk == 0 and parity == 0)
                            last = (blk == PBLK - 1 and parity == 1)
                            for dh in range(DCH):
                                nc.tensor.matmul(
                                    yT_ps[dh][:],
                                    lhsT=dec_sb[:, kc, dh * P:(dh + 1) * P],
                                    rhs=xy[:],
                                    start=first, stop=last, skip_group_check=True)
                    for dh in range(DCH):
                        nc.vector.tensor_copy(
                            out=yT_sb[:, dh, tb * 512:(tb + 1) * 512],
                            in_=yT_ps[dh][:])

            # ---------------- phase G: transpose partial y to token-major,
            # ReduceScatter over token chunks, LN only own chunk, AllGather x.
            with tc.tile_pool(name=f"tpG{layer}", bufs=2, space="PSUM") as tpps:
                for tau in range(TCH):
                    for ko in range(DCH):
                        emit_transpose(tpps, yT_sb[:, ko, tau * P:(tau + 1) * P],
                                       td_f32[:, tau, ko * P:(ko + 1) * P], id_f32, f32, "g")
            if n_cores > 1:
                with tc.tile_pool(name=f"ph{layer}", bufs=1) as ph:
                    nc.sync.dma_start(out=ar2_in.rearrange("tau p d -> p tau d"),
                                      in_=td_f32[:])
                    if cfg.get("no_cc"):
                        nc.sync.dma_start(out=ar2rs_out[:], in_=ar2_in[0:1])
                    else:
                        nc.gpsimd.collective_compute(
                            "ReduceScatter", ADD, replica_groups=all_group,
                            ins=[ar2_in.opt()], outs=[ar2rs_out.opt()])
                    ych = ph.tile([P, CHT, D], f32, name="ych")
                    nc.sync.dma_start(out=ych[:],
                                      in_=ar2rs_out.rearrange("c p d -> p c d"))
                    # z = x[own chunks] + LN(y chunk); x_next chunk = LN(z)
                    for c in range(CHT):
                        emit_ln_2d(ych[:, c, :], ych[:, c, :])
                    x_dyn = x_sb[:, ds(pid * CHT, CHT), :]
                    nc.vector.tensor_tensor(ych[:], ych[:], x_dyn, ADD)
                    for c in range(CHT):
                        emit_ln_2d(ych[:, c, :], ych[:, c, :])
                    nc.sync.dma_start(out=ar3_in.rearrange("c p d -> p c d"),
                                      in_=ych[:])
                    if cfg.get("no_cc"):
                        for i in range(TCH):
                            nc.sync.dma_start(out=ar3_out[i:i + 1], in_=ar3_in[:])
                    else:
                        nc.gpsimd.collective_compute(
                            "AllGather", mybir.AluOpType.bypass,
                            replica_groups=all_group,
                            ins=[ar3_in.opt()], outs=[ar3_out.opt()])
                    nc.sync.dma_start(out=x_sb[:],
                                      in_=ar3_out.rearrange("tau p d -> p tau d"))
            else:
                # ---------------- phase H (single-core): x = LN(x + LN(y))
                emit_ln(td_f32, td_f32)
                nc.vector.tensor_tensor(x_sb[:], x_sb[:], td_f32[:], ADD)
                emit_ln(x_sb, x_sb)

        # ---------------- logits
        with tc.tile_pool(name="tpZ", bufs=2, space="PSUM") as tpps:
            phase_A(tpps)
        with tc.tile_pool(name="lg", bufs=2) as work, \
             tc.tile_pool(name="lgps", bufs=2, space="PSUM") as ps:
            out_r = out_ap.rearrange("(tau p) v -> p tau v", p=P)
            for tau in range(TCH):
                pt = ps.tile([P, V], f32, name="lg_ps", tag="lg_ps")
                for ko in range(DCH):
                    nc.tensor.matmul(pt[:], lhsT=dmaj_bf[:, ko, tau * P:(tau + 1) * P],
                                     rhs=lm_sb[:, ko, :], start=(ko == 0), stop=(ko == DCH - 1))
                lg = work.tile([P, V], f32, name="lg_sb", tag="lg_sb")
                nc.vector.tensor_copy(out=lg[:], in_=pt[:])
                nc.sync.dma_start(out=out_r[:, tau, :], in_=lg[:])


def build(cfg):
    import concourse.bacc as bacc
    import concourse.tile as tile
    import concourse.mybir as mybir

    f32 = mybir.dt.float32
    bf = mybir.dt.bfloat16
    nc = bacc.Bacc("TRN2", target_bir_lowering=False, debug=False,
                   enable_asserts=False, num_devices=cfg["n_cores"])
    TT, HALF = cfg["t"], cfg["half"]
    in_aps = {
        "x0": nc.dram_tensor("x0", [TT, D], f32, kind="ExternalInput").ap(),
        "w_e": nc.dram_tensor("w_e", [D, HALF], bf, kind="ExternalInput").ap(),
        "w_v": nc.dram_tensor("w_v", [D, HALF], bf, kind="ExternalInput").ap(),
        "dec": nc.dram_tensor("dec", [HALF, D], bf, kind="ExternalInput").ap(),
        "lm": nc.dram_tensor("lm", [D, V], bf, kind="ExternalInput").ap(),
        "cos_t": nc.dram_tensor("cos_t", [HALF // 2, TT], bf, kind="ExternalInput").ap(),
        "sin_t": nc.dram_tensor("sin_t", [HALF // 2, TT], bf, kind="ExternalInput").ap(),
        "mask": nc.dram_tensor("mask", [P, P], f32, kind="ExternalInput").ap(),
    }
    out_ap = nc.dram_tensor("logits", [TT, V], f32, kind="ExternalOutput").ap()
    with tile.TileContext(nc) as tc:
        emit_model(tc, in_aps, out_ap, cfg)
    nc.compile()
    return nc


# ---------------------------------------------------------------- host side

def make_tables(t, n_full):
    # mirror the reference fp32 math
    n = np.arange(n_full, dtype=np.float32)
    q = np.floor(n / 2.0).astype(np.float32) * np.float32(2.0)
    base = np.power(np.float32(THETA), (q / np.float32(n_full)).astype(np.float32))
    freqs = (np.float32(1.0) / base / np.float32(2.0 * np.pi)).astype(np.float32)
    tt = np.arange(t, dtype=np.float32)[:, None]
    phases = (tt * freqs[None, :]).astype(np.float32)
    ph = ((phases % np.float32(1.0)) * np.float32(2.0 * np.pi)).astype(np.float32)
    return np.cos(ph).astype(np.float32), np.sin(ph).astype(np.float32)


def make_in_maps(idx, embed, encoder, encoder_v, decoder, lm_head, cfg):
    n_cores = cfg["n_cores"]
    half = cfg["half"]
    t = cfg["t"]
    ph_loc = half // 2
    nh = n_cores // 2
    n_full = half * 2  # per-head latent dim

    idx = np.asarray(idx).astype(np.int64)
    embed = np.asarray(embed, dtype=np.float32)
    enc = np.asarray(encoder, dtype=np.float32)
    enc_v = np.asarray(encoder_v, dtype=np.float32)
    dec = np.asarray(decoder, dtype=np.float32).reshape(nh, n_full, D)
    lm = np.asarray(lm_head, dtype=np.float32)

    x0 = embed[idx[0]].astype(np.float32)               # [t, D]
    cos_f, sin_f = make_tables(t, n_full)               # [t, n_full]
    lm_bf = lm.astype(BF16NP)
    mask = (np.arange(P)[:, None] < np.arange(P)[None, :]).astype(np.float32)

    in_maps = []
    for c in range(n_cores):
        h, j = divmod(c, 2)
        p_glob = j * ph_loc + np.arange(ph_loc)
        cols = np.concatenate([2 * p_glob, 2 * p_glob + 1])
        in_maps.append({
            "x0": x0,
            "w_e": np.ascontiguousarray(enc[h][:, cols]).astype(BF16NP),
            "w_v": np.ascontiguousarray(enc_v[h][:, cols]).astype(BF16NP),
            "dec": np.ascontiguousarray(dec[h][cols, :]).astype(BF16NP),
            "lm": lm_bf,
            "cos_t": np.ascontiguousarray(cos_f[:, 2 * p_glob].T).astype(BF16NP),
            "sin_t": np.ascontiguousarray(sin_f[:, 2 * p_glob].T).astype(BF16NP),
            "mask": mask,
        })
    return in_maps


def _get_nc(cfg_key=None, cfg=None):
    if cfg is None:
        cfg = _cfg()
    key = tuple(sorted(cfg.items()))
    if key not in _CACHE:
        _CACHE[key] = build(cfg)
    return _CACHE[key]


def run(inputs, cfg=None, trace=False, **run_kwargs):
    from concourse.bass_utils import run_bass_kernel_spmd
    if cfg is None:
        cfg = _cfg()
    nc = _get_nc(cfg=cfg)
    in_maps = make_in_maps(inputs["idx"], inputs["embed"], inputs["encoder"],
                           inputs["encoder_v"], inputs["decoder"],
                           inputs["lm_head"], cfg)
    res = run_bass_kernel_spmd(nc, in_maps, core_ids=list(range(cfg["n_cores"])),
                               trace=trace, **run_kwargs)
    logits = np.asarray(res.results[0]["logits"], dtype=np.float32)
    return logits.reshape(1, cfg["t"], V), res


def kernel(idx, embed, encoder, encoder_v, decoder, lm_head):
    logits, _ = run(dict(idx=idx, embed=embed, encoder=encoder,
                         encoder_v=encoder_v, decoder=decoder, lm_head=lm_head))
    return logits



# revision 13
# speedup vs baseline: 1.2070x; 1.2070x over previous
"""BDH dense-transformer Trainium2 kernel (8 NeuronCores, SPMD).

Model (weight-tied, 4 layers): T=1024, D=256, NH=4, N=8192/head, VOCAB=256.

Sharding: core c -> head h=c//2, latent half j=c%2 (4096 latent dims/core).
  - encoder/encoder_v column-sharded, decoder row-sharded (host-permuted so
    rope pairs are de-interleaved: local m in [0,2048) = even pair elements,
    [2048,4096) = odd elements; permutation applied consistently to all three
    weight shards so scores/gate/decoder are unaffected).
  - scores trick: yKV = mask(qr qr^T) @ x distributes over latent shards:
    each core computes mask(qr_loc qr_loc^T) @ x, pairwise AllReduce on yKV.
  - decoder partial sums: 8-way AllReduce on y (bf16), LN/residual fully local.

Layer is a software pipeline over two 512-token blocks (tb=0,1):
  B(tb):   x_sp(tb)=relu(W_e^T xT(tb)), rope -> qr(tb), spill x_sp; score
           tiles whose rhs lives in tb accumulate lagged inside the loop
           (sc0 = scores cols tb0, si 0..3; sc1a = scores cols tb1, si 0..3).
  sc1b:    scores cols tb1, si 4..7 (needs qr(tb1) rows).
  yKV(tb): masked scores @ x -> pairwise AllReduce in bf16 (AR1).
  F(tb):   y_sp=relu(W_v^T yKVlnT), gate with reloaded x_sp, decoder partial.
  ARy(tb): 8-way AllReduce of partial y (bf16), then local LN+residual+LN.
  xupd(tb) feeds the next layer's B(tb); xupd1(L-1) is emitted after B0(L) so
  its AR-wait never head-of-line-blocks B0's DVE/PE streams.
Collectives (and the DMAs directly feeding/draining them) are the only thing
on the Pool queue, so collective waits never block compute issue.

On-device layouts: big tensors feature-major [latent, T] (contraction dims on
partitions); residual stream x token-major [T, D] fp32; matmuls bf16 with
fp32 PSUM accumulation.
"""

import os
import numpy as np
import ml_dtypes

BF16NP = ml_dtypes.bfloat16

# full-size config
T = 1024
D = 256
NH = 4
N = 8192
V = 256
L = 4
NCORES = 8
P = 128
EPS = 1e-5
THETA = 2.0 ** 16

_CACHE = {}


def _cfg(n_cores=NCORES, half=N // 2, t=T, layers=L, no_cc=False):
    # half: latent width per core (= N*NH/n_cores)
    assert t % 512 == 0 and half % 256 == 0
    return dict(
        n_cores=n_cores, half=half, t=t, layers=layers, no_cc=no_cc,
        tch=t // P,            # token chunks
        kch=half // P,         # latent chunks
        pblk=half // 2 // P,   # pair blocks (even/odd chunk pairs)
        tb_n=t // 512,         # 512-wide t blocks
        dch=D // P,            # 2
    )


# ---------------------------------------------------------------- device code

def emit_model(tc, in_aps, out_ap, cfg):
    from contextlib import ExitStack
    import concourse.mybir as mybir
    from concourse.masks import make_identity

    nc = tc.nc
    f32 = mybir.dt.float32
    bf = mybir.dt.bfloat16
    ADD = mybir.AluOpType.add
    SUB = mybir.AluOpType.subtract
    MULT = mybir.AluOpType.mult
    Relu = mybir.ActivationFunctionType.Relu
    Sqrt = mybir.ActivationFunctionType.Sqrt

    n_cores = cfg["n_cores"]
    TCH, KCH, PBLK, TB, DCH = (cfg[k] for k in ("tch", "kch", "pblk", "tb_n", "dch"))
    TT = cfg["t"]
    HALF = cfg["half"]
    layers = cfg["layers"]
    no_cc = cfg.get("no_cc", False)
    pair_groups = [[2 * i, 2 * i + 1] for i in range(max(n_cores // 2, 1))]
    all_group = [list(range(n_cores))]
    NSB = 4 * TB          # score row strips for the last t-block

    with ExitStack() as ctx:
        singles = ctx.enter_context(tc.tile_pool(name="singles", bufs=1))
        state = ctx.enter_context(tc.tile_pool(name="state", bufs=1))
        dram = ctx.enter_context(tc.tile_pool(name="dramp", bufs=1, space="DRAM"))
        lnp = ctx.enter_context(tc.tile_pool(name="lnp", bufs=4))

        # ---- resident constants (x0 is DMA'd first, below; w_e next: B0
        # depends only on those, and the x0 LN chain overlaps the w_e DMA)
        we_sb = singles.tile([P, DCH, HALF], bf, name="we_sb")
        nc.sync.dma_start(out=we_sb[:], in_=in_aps["w_e"].rearrange("(ko p) m -> p ko m", p=P))
        mask_sb = singles.tile([P, P], f32, name="mask_sb")
        nc.gpsimd.dma_start(out=mask_sb[:], in_=in_aps["mask"][:])
        wv_sb = singles.tile([P, DCH, HALF], bf, name="wv_sb")
        nc.gpsimd.dma_start(out=wv_sb[:], in_=in_aps["w_v"].rearrange("(ko p) m -> p ko m", p=P))
        dec_sb = singles.tile([P, KCH, D], bf, name="dec_sb")
        nc.gpsimd.dma_start(out=dec_sb[:], in_=in_aps["dec"].rearrange("(kc p) d -> p kc d", p=P))
        lm_sb = singles.tile([P, DCH, V], bf, name="lm_sb")
        nc.gpsimd.dma_start(out=lm_sb[:], in_=in_aps["lm"].rearrange("(ko p) v -> p ko v", p=P))
        eps_sb = singles.tile([P, 1], f32, name="eps_sb")
        nc.vector.memset(eps_sb[:], EPS)
        id_bf = singles.tile([P, P], bf, name="id_bf")
        make_identity(nc, id_bf[:])

        # ---- resident state
        x_sb = state.tile([P, TCH, D], f32, name="x_sb")        # residual, token-major
        tmaj_bf = state.tile([P, TCH, D], bf, name="tmaj_bf")   # bf16 copy of x
        ykvln_bf = state.tile([P, TCH, D], bf, name="ykvln_bf")  # LN(yKV) bf16
        dmaj_bf = state.tile([P, DCH, TT], bf, name="dmaj_bf")  # xT / yKV_lnT
        qr_sb = state.tile([P, KCH, TT], bf, name="qr_sb")
        td_f32 = state.tile([P, TCH, D], f32, name="td_f32")    # x0 / LN scratch
        yt_bf = state.tile([P, DCH, TT], bf, name="yt_bf")      # partial y feature-major
        # collective staging (token-major bf16)
        yk_stage = [state.tile([P, 4, D], bf, name=f"yk_stage{tb}") for tb in range(TB)]
        ykv_post = [state.tile([P, 4, D], bf, name=f"ykv_post{tb}") for tb in range(TB)]
        y_stage = [state.tile([P, 4, D], bf, name=f"y_stage{tb}") for tb in range(TB)]
        y_post = [state.tile([P, 4, D], bf, name=f"y_post{tb}") for tb in range(TB)]

        # ---- dram scratch
        xsp_dram = dram.tile([P, PBLK, 2, TT], bf, name="xsp_dram")
        ar1_ins = [dram.tile([P, 4, D], bf, name=f"ar1_in{tb}") for tb in range(TB)]
        ar1_outs = [dram.tile([P, 4, D], bf, name=f"ar1_out{tb}") for tb in range(TB)]
        ary_ins = [dram.tile([P, 4, D], bf, name=f"ary_in{tb}") for tb in range(TB)]
        ary_outs = [dram.tile([P, 4, D], bf, name=f"ary_out{tb}") for tb in range(TB)]
        rs_in = dram.tile([TCH, P, D], bf, name="rs_in")
        rs_out = dram.tile([1, P, D], bf, name="rs_out")

        def emit_ln_2d(src2, dst2):
            # LayerNorm over D of one [P, D] tile
            stats = lnp.tile([P, 6], f32, name="ln_stats", tag="ln_stats")
            nc.vector.bn_stats(out=stats[:], in_=src2)
            mv = lnp.tile([P, 2], f32, name="ln_mv", tag="ln_mv")
            nc.vector.bn_aggr(out=mv[:], in_=stats[:])
            std = lnp.tile([P, 1], f32, name="ln_std", tag="ln_std")
            nc.scalar.activation(out=std[:], in_=mv[:, 1:2], func=Sqrt,
                                 bias=eps_sb[:, 0:1])
            rstd = lnp.tile([P, 1], f32, name="ln_rstd", tag="ln_rstd")
            nc.vector.reciprocal(out=rstd[:], in_=std[:])
            nc.vector.tensor_scalar(out=dst2, in0=src2,
                                    scalar1=mv[:, 0:1], scalar2=rstd[:],
                                    op0=SUB, op1=MULT)

        def cc(kind, op, groups, in_t, out_t):
            if no_cc:
                nc.gpsimd.dma_start(out=out_t[:], in_=in_t[:])
            else:
                nc.gpsimd.collective_compute(
                    kind, op, replica_groups=groups,
                    ins=[in_t.opt()], outs=[out_t.opt()])

        def emit_transpose(ps_pool, src2, dst2, tag):
            pt = ps_pool.tile([P, P], bf, name=f"tp_{tag}", tag=f"tp_{tag}")
            nc.tensor.transpose(pt[:], src2, id_bf[:])
            nc.vector.tensor_copy(out=dst2, in_=pt[:])

        def emit_B(layer, tb, work, xspP, emit_sc):
            """x_sp/rope/spill for t-block tb; emit_sc(blk) emits the lagged
            score accumulation whose rhs lives in this t-block."""
            LAG = 2
            c0, c1 = tb * 512, (tb + 1) * 512

            for blk in range(PBLK + LAG):
                if blk < PBLK:
                    cos_sb = work.tile([P, 512], bf, name="cos_sb", tag="cos")
                    nc.sync.dma_start(out=cos_sb[:],
                                      in_=in_aps["cos_t"][blk * P:(blk + 1) * P, c0:c1])
                    sin_sb = work.tile([P, 512], bf, name="sin_sb", tag="sin")
                    nc.sync.dma_start(out=sin_sb[:],
                                      in_=in_aps["sin_t"][blk * P:(blk + 1) * P, c0:c1])
                    vb = work.tile([P, 2, 512], bf, name="vb", tag="vb")
                    for parity in (0, 1):
                        kc = blk + PBLK * parity
                        pt = xspP.tile([P, 512], f32, name="xsp_ps", tag="xsp_ps")
                        for ko in range(DCH):
                            nc.tensor.matmul(
                                pt[:], lhsT=we_sb[:, ko, kc * P:(kc + 1) * P],
                                rhs=dmaj_bf[:, ko, c0:c1],
                                start=(ko == 0), stop=(ko == DCH - 1))
                        nc.scalar.activation(out=vb[:, parity, :], in_=pt[:], func=Relu)
                    nc.sync.dma_start(out=xsp_dram[:, blk, :, c0:c1], in_=vb[:])
                    # rope (all on DVE; Pool is reserved for collectives)
                    ve, vo = vb[:, 0, :], vb[:, 1, :]
                    t1 = work.tile([P, 512], bf, name="t1", tag="t1")
                    t2 = work.tile([P, 512], bf, name="t2", tag="t2")
                    nc.vector.tensor_mul(out=t1[:], in0=ve, in1=cos_sb[:])
                    nc.vector.tensor_mul(out=t2[:], in0=vo, in1=sin_sb[:])
                    nc.vector.tensor_tensor(qr_sb[:, blk, c0:c1], t1[:], t2[:], SUB)
                    t3 = work.tile([P, 512], bf, name="t3", tag="t1")
                    t4 = work.tile([P, 512], bf, name="t4", tag="t2")
                    nc.vector.tensor_mul(out=t3[:], in0=vo, in1=cos_sb[:])
                    nc.vector.tensor_mul(out=t4[:], in0=ve, in1=sin_sb[:])
                    nc.vector.tensor_tensor(qr_sb[:, blk + PBLK, c0:c1], t3[:], t4[:], ADD)
                if blk >= LAG:
                    emit_sc(blk - LAG)

        # diagonal score blocks (rows si == cols sub) are triangle-trimmed at
        # 256-col granularity: strip j keeps col halves h with (h+1)*256 > j*128
        def diag_halves(j):
            return (0, 1) if j < 2 else (1,)

        def make_diag_ps(pool, pfx):
            return {(j, h): pool.tile([P, 256], f32, name=f"{pfx}_{j}_{h}",
                                      tag=f"{pfx}_{j}_{h}", bufs=1)
                    for j in range(4) for h in diag_halves(j)}

        def emit_diag_sc(sc_ps, blk, tbc):
            # lagged accumulation of diagonal strips; rhs = qr cols of tbc
            for j in range(4):
                for h in diag_halves(j):
                    for kc in (blk, blk + PBLK):
                        nc.tensor.matmul(
                            sc_ps[(j, h)][:],
                            lhsT=qr_sb[:, kc, (4 * tbc + j) * P:(4 * tbc + j + 1) * P],
                            rhs=qr_sb[:, kc, tbc * 512 + h * 256:tbc * 512 + (h + 1) * 256],
                            start=(blk == 0 and kc == blk),
                            stop=(blk == PBLK - 1 and kc == blk + PBLK),
                            skip_group_check=True)

        def emit_diag_mask(sc_ps, sc_sb, si_base):
            # mask/copy trimmed diagonal strips into sc_sb rows si_base+j;
            # local cols 0..512 of the t-block
            for j in range(4):
                for h in diag_halves(j):
                    pt = sc_ps[(j, h)]
                    lo = h * 256
                    if lo <= j * P < lo + 256:
                        o = j * P - lo
                        nc.vector.tensor_mul(
                            out=sc_sb[:, si_base + j, j * P:(j + 1) * P],
                            in0=pt[:, o:o + P], in1=mask_sb[:])
                        if o + P < 256:
                            nc.vector.tensor_copy(
                                out=sc_sb[:, si_base + j, lo + o + P:lo + 256],
                                in_=pt[:, o + P:256])
                    else:
                        nc.vector.tensor_copy(
                            out=sc_sb[:, si_base + j, lo:lo + 256], in_=pt[:])

        def emit_ykv(tb, sc_sb, ykP):
            # yKV partials for taus of tb -> yk_stage[tb] (bf16)
            for sub in range(4):
                tau = tb * 4 + sub
                yk = ykP.tile([P, D], f32, name="yk_ps", tag="yk_ps")
                for si in range(tau + 1):
                    nc.tensor.matmul(
                        yk[:], lhsT=sc_sb[:, si, sub * P:(sub + 1) * P],
                        rhs=tmaj_bf[:, si, :],
                        start=(si == 0), stop=(si == tau))
                nc.vector.tensor_copy(out=yk_stage[tb][:, sub, :], in_=yk[:])

        def emit_ar1(tb):
            nc.gpsimd.dma_start(out=ar1_ins[tb][:], in_=yk_stage[tb][:])
            if n_cores > 1:
                cc("AllReduce", ADD, pair_groups, ar1_ins[tb], ar1_outs[tb])
            else:
                nc.gpsimd.dma_start(out=ar1_outs[tb][:], in_=ar1_ins[tb][:])
            nc.gpsimd.dma_start(out=ykv_post[tb][:], in_=ar1_outs[tb][:])

        def emit_ln_etp(tb, tpps):
            # LN(yKV) for taus of tb, then transpose into dmaj columns of tb
            for sub in range(4):
                tau = tb * 4 + sub
                emit_ln_2d(ykv_post[tb][:, sub, :], ykvln_bf[:, tau, :])
            for sub in range(4):
                tau = tb * 4 + sub
                for ko in range(DCH):
                    emit_transpose(tpps, ykvln_bf[:, tau, ko * P:(ko + 1) * P],
                                   dmaj_bf[:, ko, tau * P:(tau + 1) * P], "e")

        def emit_F(layer, tb, work, psY, psF, tpps):
            c0, c1 = tb * 512, (tb + 1) * 512
            yT_ps = [psY.tile([P, 512], f32, name=f"yt_ps{dh}", tag=f"yt_ps{dh}")
                     for dh in range(DCH)]
            for blk in range(PBLK):
                xsp = work.tile([P, 2, 512], bf, name="xsp_r", tag="xsp_r")
                nc.sync.dma_start(out=xsp[:], in_=xsp_dram[:, blk, :, c0:c1])
                for parity in (0, 1):
                    kc = blk + PBLK * parity
                    ysp = work.tile([P, 512], bf, name="ysp", tag="ysp")
                    pt = psF.tile([P, 512], f32, name="ysp_ps", tag="ysp_ps")
                    for ko in range(DCH):
                        nc.tensor.matmul(
                            pt[:], lhsT=wv_sb[:, ko, kc * P:(kc + 1) * P],
                            rhs=dmaj_bf[:, ko, c0:c1],
                            start=(ko == 0), stop=(ko == DCH - 1))
                    nc.scalar.activation(out=ysp[:], in_=pt[:], func=Relu)
                    xy = work.tile([P, 512], bf, name="xy", tag="xy")
                    nc.vector.tensor_mul(out=xy[:], in0=xsp[:, parity, :], in1=ysp[:])
                    first = (blk == 0 and parity == 0)
                    last = (blk == PBLK - 1 and parity == 1)
                    for dh in range(DCH):
                        nc.tensor.matmul(
                            yT_ps[dh][:],
                            lhsT=dec_sb[:, kc, dh * P:(dh + 1) * P],
                            rhs=xy[:],
                            start=first, stop=last, skip_group_check=True)
            for dh in range(DCH):
                nc.vector.tensor_copy(out=yt_bf[:, dh, c0:c1], in_=yT_ps[dh][:])
            # transpose partial y to token-major bf16 stage
            for sub in range(4):
                tau = tb * 4 + sub
                for dh in range(DCH):
                    emit_transpose(tpps, yt_bf[:, dh, tau * P:(tau + 1) * P],
                                   y_stage[tb][:, sub, dh * P:(dh + 1) * P], "g")

        def emit_ary(tb):
            nc.gpsimd.dma_start(out=ary_ins[tb][:], in_=y_stage[tb][:])
            if n_cores > 1:
                cc("AllReduce", ADD, all_group, ary_ins[tb], ary_outs[tb])
            else:
                nc.gpsimd.dma_start(out=ary_outs[tb][:], in_=ary_ins[tb][:])
            nc.gpsimd.dma_start(out=y_post[tb][:], in_=ary_outs[tb][:])

        def emit_xupd(tb, tpps):
            # x = LN(x + LN(y)) for taus of tb; refresh tmaj/dmaj
            for sub in range(4):
                tau = tb * 4 + sub
                emit_ln_2d(y_post[tb][:, sub, :], td_f32[:, tau, :])
                nc.vector.tensor_tensor(td_f32[:, tau, :], td_f32[:, tau, :],
                                        x_sb[:, tau, :], ADD)
                emit_ln_2d(td_f32[:, tau, :], x_sb[:, tau, :])
                nc.vector.tensor_copy(out=tmaj_bf[:, tau, :], in_=x_sb[:, tau, :])
                for ko in range(DCH):
                    emit_transpose(tpps, tmaj_bf[:, tau, ko * P:(ko + 1) * P],
                                   dmaj_bf[:, ko, tau * P:(tau + 1) * P], "a")

        out_r = out_ap.rearrange("(tau p) v -> p tau v", p=P)

        def emit_final(pid):
            # last layer: single ReduceScatter of the full y; each core LNs,
            # adds its own residual chunk, and emits logits for its own 128
            # tokens into output rows [0:P] (host reassembles across cores).
            from concourse.bass import ds
            for tb in range(TB):
                nc.gpsimd.dma_start(
                    out=rs_in[tb * 4:(tb + 1) * 4].rearrange("c p d -> p c d"),
                    in_=y_stage[tb][:])
            if n_cores > 1 and not no_cc:
                nc.gpsimd.collective_compute(
                    "ReduceScatter", ADD, replica_groups=all_group,
                    ins=[rs_in.opt()], outs=[rs_out.opt()])
            else:
                nc.gpsimd.dma_start(out=rs_out[:], in_=rs_in[0:1])
            with tc.tile_pool(name="fin", bufs=1) as fw, \
                 tc.tile_pool(name="finps", bufs=2, space="PSUM") as ps:
                ych = fw.tile([P, 1, D], bf, name="ych")
                nc.gpsimd.dma_start(out=ych[:], in_=rs_out.rearrange("c p d -> p c d"))
                yln = fw.tile([P, 1, D], f32, name="yln")
                emit_ln_2d(ych[:, 0, :], yln[:, 0, :])
                if pid is not None:
                    x_dyn = x_sb[:, ds(pid, 1), :]
                else:
                    x_dyn = x_sb[:, 0:1, :]
                nc.vector.tensor_tensor(yln[:], yln[:], x_dyn, ADD)
                xf = fw.tile([P, D], bf, name="xf")
                emit_ln_2d(yln[:, 0, :], xf[:])
                xfT = fw.tile([P, D], bf, name="xfT")
                for ko in range(DCH):
                    emit_transpose(ps, xf[:, ko * P:(ko + 1) * P],
                                   xfT[:, ko * P:(ko + 1) * P], "z")
                pt = ps.tile([P, V], f32, name="lg_ps", tag="lg_ps")
                for ko in range(DCH):
                    nc.tensor.matmul(pt[:], lhsT=xfT[:, ko * P:(ko + 1) * P],
                                     rhs=lm_sb[:, ko, :], start=(ko == 0),
                                     stop=(ko == DCH - 1))
                lg = fw.tile([P, V], f32, name="lg_sb")
                nc.vector.tensor_copy(out=lg[:], in_=pt[:])
                nc.sync.dma_start(out=out_r[:, 0, :], in_=lg[:])

        # ---- initial: x = LN(x0); tmaj/dmaj per t-block
        pid = nc.vector.partition_id() if n_cores > 1 else None
        nc.scalar.dma_start(out=td_f32[:], in_=in_aps["x0"].rearrange("(tau p) d -> p tau d", p=P))
        with tc.tile_pool(name="tpI", bufs=2, space="PSUM") as tpps:
            for tau in range(TCH):
                emit_ln_2d(td_f32[:, tau, :], x_sb[:, tau, :])
            nc.vector.tensor_copy(out=tmaj_bf[:], in_=x_sb[:])
            for tau in range(TCH):
                for ko in range(DCH):
                    emit_transpose(tpps, tmaj_bf[:, tau, ko * P:(ko + 1) * P],
                                   dmaj_bf[:, ko, tau * P:(tau + 1) * P], "i")

        for layer in range(layers):
            last = layer == layers - 1
            scw_cm = tc.tile_pool(name=f"scb_{layer}", bufs=1)
            scw = scw_cm.__enter__()
            # ---------------- B0 (+ lagged trimmed sc0)
            sc0_cm = tc.tile_pool(name=f"sc0_{layer}", bufs=1, space="PSUM")
            sc0 = sc0_cm.__enter__()
            sc0_ps = make_diag_ps(sc0, "sc0")
            with tc.tile_pool(name=f"b0_{layer}", bufs=3) as work, \
                 tc.tile_pool(name=f"b0ps{layer}", bufs=2, space="PSUM") as xspP:
                emit_B(layer, 0, work, xspP, lambda blk: emit_diag_sc(sc0_ps, blk, 0))

            # ---------------- mask0 + yKV0 + AR1(0)  (before xupd1 so AR1(0)
            # is never gated by the previous layer's tb1 LN chain)
            sc_sb0 = scw.tile([P, 4, 512], bf, name="sc_sb0", tag="sc_sb0")
            emit_diag_mask(sc0_ps, sc_sb0, 0)
            sc0_cm.__exit__(None, None, None)
            with tc.tile_pool(name=f"yk0_{layer}", bufs=2, space="PSUM") as ykP:
                emit_ykv(0, sc_sb0, ykP)
            emit_ar1(0)

            # ---------------- xupd1 of previous layer (feeds B1)
            if layer > 0:
                with tc.tile_pool(name=f"tpU1_{layer}", bufs=2, space="PSUM") as tpps:
                    emit_xupd(1, tpps)

            # ---------------- B1 (+ lagged sc1a) ; sc1b ; mask1 ; yKV1
            sc1a_cm = tc.tile_pool(name=f"sc1a_{layer}", bufs=1, space="PSUM")
            sc1a = sc1a_cm.__enter__()
            sc1a_ps = [sc1a.tile([P, 512], f32, name=f"sc1a_{si}",
                                 tag=f"sc1a_{si}", bufs=1) for si in range(4)]

            def emit_sc1a(blk, _ps=sc1a_ps):
                for si in range(4):
                    for kc in (blk, blk + PBLK):
                        nc.tensor.matmul(
                            _ps[si][:], lhsT=qr_sb[:, kc, si * P:(si + 1) * P],
                            rhs=qr_sb[:, kc, 512:1024],
                            start=(blk == 0 and kc == blk),
                            stop=(blk == PBLK - 1 and kc == blk + PBLK),
                            skip_group_check=True)

            with tc.tile_pool(name=f"b1_{layer}", bufs=3) as work, \
                 tc.tile_pool(name=f"b1ps{layer}", bufs=2, space="PSUM") as xspP:
                emit_B(layer, 1, work, xspP, emit_sc1a)
            sc_sb1 = scw.tile([P, 8, 512], bf, name="sc_sb1", tag="sc_sb1")
            # sc1a strips copy while sc1b computes (DVE overlaps PE)
            for si in range(4):
                nc.vector.tensor_copy(out=sc_sb1[:, si, :], in_=sc1a_ps[si][:])
            sc1a_cm.__exit__(None, None, None)
            sc1b_cm = tc.tile_pool(name=f"sc1b_{layer}", bufs=1, space="PSUM")
            sc1b = sc1b_cm.__enter__()
            sc1b_ps = make_diag_ps(sc1b, "sc1b")
            for blk in range(PBLK):
                emit_diag_sc(sc1b_ps, blk, 1)
            emit_diag_mask(sc1b_ps, sc_sb1, 4)
            sc1b_cm.__exit__(None, None, None)
            with tc.tile_pool(name=f"yk1_{layer}", bufs=2, space="PSUM") as ykP:
                emit_ykv(1, sc_sb1, ykP)
            emit_ar1(1)

            # ---------------- LN0/Etp0 -> F0 -> ARy(0)
            with tc.tile_pool(name=f"e0_{layer}", bufs=2, space="PSUM") as tpE, \
                 tc.tile_pool(name=f"f0_{layer}", bufs=3) as work, \
                 tc.tile_pool(name=f"f0y{layer}", bufs=1, space="PSUM") as psY, \
                 tc.tile_pool(name=f"f0s{layer}", bufs=2, space="PSUM") as psF:
                emit_ln_etp(0, tpE)
                emit_F(layer, 0, work, psY, psF, tpE)
                if not last:
                    emit_ary(0)

            # ---------------- LN1/Etp1 -> F1 -> ARy(1)
            with tc.tile_pool(name=f"e1_{layer}", bufs=2, space="PSUM") as tpE, \
                 tc.tile_pool(name=f"f1_{layer}", bufs=3) as work, \
                 tc.tile_pool(name=f"f1y{layer}", bufs=1, space="PSUM") as psY, \
                 tc.tile_pool(name=f"f1s{layer}", bufs=2, space="PSUM") as psF:
                emit_ln_etp(1, tpE)
                emit_F(layer, 1, work, psY, psF, tpE)
                if not last:
                    emit_ary(1)

            if last:
                emit_final(pid)
            else:
                # ---------------- xupd0 (tb0 residual update; feeds next B0)
                with tc.tile_pool(name=f"tpU0_{layer}", bufs=2, space="PSUM") as tpps:
                    emit_xupd(0, tpps)
            scw_cm.__exit__(None, None, None)


def build(cfg):
    import concourse.bacc as bacc
    import concourse.tile as tile
    import concourse.mybir as mybir

    f32 = mybir.dt.float32
    bf = mybir.dt.bfloat16
    nc = bacc.Bacc("TRN2", target_bir_lowering=False, debug=False,
                   enable_asserts=False, num_devices=cfg["n_cores"])
    TT, HALF = cfg["t"], cfg["half"]
    in_aps = {
        "x0": nc.dram_tensor("x0", [TT, D], f32, kind="ExternalInput").ap(),
        "w_e": nc.dram_tensor("w_e", [D, HALF], bf, kind="ExternalInput").ap(),
        "w_v": nc.dram_tensor("w_v", [D, HALF], bf, kind="ExternalInput").ap(),
        "dec": nc.dram_tensor("dec", [HALF, D], bf, kind="ExternalInput").ap(),
        "lm": nc.dram_tensor("lm", [D, V], bf, kind="ExternalInput").ap(),
        "cos_t": nc.dram_tensor("cos_t", [HALF // 2, TT], bf, kind="ExternalInput").ap(),
        "sin_t": nc.dram_tensor("sin_t", [HALF // 2, TT], bf, kind="ExternalInput").ap(),
        "mask": nc.dram_tensor("mask", [P, P], f32, kind="ExternalInput").ap(),
    }
    out_ap = nc.dram_tensor("logits", [TT, V], f32, kind="ExternalOutput").ap()
    with tile.TileContext(nc) as tc:
        emit_model(tc, in_aps, out_ap, cfg)
    nc.compile()
    return nc


# ---------------------------------------------------------------- host side

def make_tables(t, n_full):
    # mirror the reference fp32 math
    n = np.arange(n_full, dtype=np.float32)
    q = np.floor(n / 2.0).astype(np.float32) * np.float32(2.0)
    base = np.power(np.float32(THETA), (q / np.float32(n_full)).astype(np.float32))
    freqs = (np.float32(1.0) / base / np.float32(2.0 * np.pi)).astype(np.float32)
    tt = np.arange(t, dtype=np.float32)[:, None]
    phases = (tt * freqs[None, :]).astype(np.float32)
    ph = ((phases % np.float32(1.0)) * np.float32(2.0 * np.pi)).astype(np.float32)
    return np.cos(ph).astype(np.float32), np.sin(ph).astype(np.float32)


def make_in_maps(idx, embed, encoder, encoder_v, decoder, lm_head, cfg):
    n_cores = cfg["n_cores"]
    half = cfg["half"]
    t = cfg["t"]
    ph_loc = half // 2
    nh = max(n_cores // 2, 1)
    n_full = half * 2  # per-head latent dim

    idx = np.asarray(idx).astype(np.int64)
    embed = np.asarray(embed, dtype=np.float32)
    enc = np.asarray(encoder, dtype=np.float32)
    enc_v = np.asarray(encoder_v, dtype=np.float32)
    dec = np.asarray(decoder, dtype=np.float32).reshape(nh, n_full, D)
    lm = np.asarray(lm_head, dtype=np.float32)

    x0 = embed[idx[0]].astype(np.float32)               # [t, D]
    cos_f, sin_f = make_tables(t, n_full)               # [t, n_full]
    lm_bf = lm.astype(BF16NP)
    mask = (np.arange(P)[:, None] < np.arange(P)[None, :]).astype(np.float32)

    in_maps = []
    for c in range(n_cores):
        h, j = divmod(c, 2)
        p_glob = j * ph_loc + np.arange(ph_loc)
        cols = np.concatenate([2 * p_glob, 2 * p_glob + 1])
        in_maps.append({
            "x0": x0,
            "w_e": np.ascontiguousarray(enc[h][:, cols]).astype(BF16NP),
            "w_v": np.ascontiguousarray(enc_v[h][:, cols]).astype(BF16NP),
            "dec": np.ascontiguousarray(dec[h][cols, :]).astype(BF16NP),
            "lm": lm_bf,
            "cos_t": np.ascontiguousarray(cos_f[:, 2 * p_glob].T).astype(BF16NP),
            "sin_t": np.ascontiguousarray(sin_f[:, 2 * p_glob].T).astype(BF16NP),
            "mask": mask,
        })
    return in_maps


def _get_nc(cfg_key=None, cfg=None):
    if cfg is None:
        cfg = _cfg()
    key = tuple(sorted(cfg.items()))
    if key not in _CACHE:
        _CACHE[key] = build(cfg)
    return _CACHE[key]


def run(inputs, cfg=None, trace=False, **run_kwargs):
    from concourse.bass_utils import run_bass_kernel_spmd
    if cfg is None:
        cfg = _cfg()
    nc = _get_nc(cfg=cfg)
    in_maps = make_in_maps(inputs["idx"], inputs["embed"], inputs["encoder"],
                           inputs["encoder_v"], inputs["decoder"],
                           inputs["lm_head"], cfg)
    res = run_bass_kernel_spmd(nc, in_maps, core_ids=list(range(cfg["n_cores"])),
                               trace=trace, **run_kwargs)
    # each core writes logits for its own 128-token chunk into rows [0:P]
    logits = np.concatenate(
        [np.asarray(res.results[c]["logits"][:P], dtype=np.float32)
         for c in range(cfg["n_cores"])], axis=0)
    return logits.reshape(1, cfg["t"], V), res


def kernel(idx, embed, encoder, encoder_v, decoder, lm_head):
    logits, _ = run(dict(idx=idx, embed=embed, encoder=encoder,
                         encoder_v=encoder_v, decoder=decoder, lm_head=lm_head))
    return logits
